# revision 19
# baseline (speedup 1.0000x reference)
"""Trainium2 Bass kernel for nn_DecoderRNN (Bahdanau attention + GRU cell +
LayerNorm + vocab projection + log-softmax), SPMD over 8 NeuronCores.

Sharding: batch (N=256 -> 32/core) for attention/GRU; vocab (V=50257 ->
6656/core padded) for the H->V projection; one AllGather of the hidden state
between the phases and one tiny AllGather for the log-softmax statistics.
"""
from contextlib import ExitStack

import numpy as np
import ml_dtypes

import concourse.bass as bass
import concourse.tile as tile
from concourse import bacc, mybir
from concourse import bass_utils
from concourse.masks import make_identity

BF16 = mybir.dt.bfloat16
F32 = mybir.dt.float32
AF = mybir.ActivationFunctionType
ALU = mybir.AluOpType
AX = mybir.AxisListType

# Model dims (hardcoded per problem spec)
V, E, H, N, S = 50257, 512, 1024, 256, 128
EPS = 1e-5
NC = 8            # cores
NB = N // NC      # batch rows per core = 32
VS = 6656         # vocab shard (13 * 512), 8*6656 = 53248 >= V
VCH = VS // 512   # 13 v-chunks
KH = H // 128     # 8 k-tiles over H
KE = E // 128     # 4 k-tiles over E
SC = 8            # score chunks: (s,n) = 4096 -> 8 chunks of 512 (16 s each)
NEG_BIG = -1e30

bf16 = ml_dtypes.bfloat16


def build_program():
    nc = bacc.Bacc("TRN2", target_bir_lowering=False, debug=False, num_devices=NC)

    def din(name, shape, dt=BF16):
        return nc.dram_tensor(name, shape, dt, kind="ExternalInput").ap()

    T = {}
    # --- external inputs (per core) ---
    T["encT"] = din("encT", (H, NB, S))              # enc transposed, bf16
    T["hidT"] = din("hidT", (H, NB))                 # hidden transposed, bf16
    T["hid32"] = din("hid32", (NB, H), F32)          # hidden natural, f32
    T["xT"] = din("xT", (E, NB))                     # emb rows transposed, bf16
    T["Uw"] = din("Uw", (H, H))
    T["Ww"] = din("Ww", (H, H))
    T["o1w"] = din("o1w", (H, H))
    T["combw"] = din("combw", (H + E, E))
    T["ihw"] = din("ihw", (E, 2 * H))
    T["hhw"] = din("hhw", (H, 2 * H))
    T["candw"] = din("candw", (E, H))
    T["hhcw"] = din("hhcw", (H, H))
    T["w2"] = din("w2", (H, VS))
    T["vT"] = din("vT", (128, KH))
    T["btanh"] = din("btanh", (128, KH), F32)        # W_b + U_b, tiled
    T["o1bT"] = din("o1bT", (128, KH), F32)
    T["combbT"] = din("combbT", (128, KE), F32)
    T["gateb"] = din("gateb", (1, 2 * H))       # ih_b + hh_b
    T["candb"] = din("candb", (1, H))
    T["hhcb"] = din("hhcb", (1, H))
    T["lng"] = din("lng", (1, H))
    T["lnb"] = din("lnb", (1, H))
    T["w2b"] = din("w2b", (1, VS))              # padded with -1e30
    T["sel"] = din("sel", (128, NB * S))        # Wh broadcast selector (zero-padded K)

    # --- external outputs (per core) ---
    T["out_lg"] = nc.dram_tensor("out_lg", (N, VS), F32, kind="ExternalOutput").ap()
    T["out_h"] = nc.dram_tensor("out_h", (NB, H), F32, kind="ExternalOutput").ap()
    T["out_attn"] = nc.dram_tensor("out_attn", (S, NB), F32,
                                   kind="ExternalOutput").ap()

    with tile.TileContext(nc) as tc:
        _build_tile(nc, tc, T)
    nc.compile()
    return nc


def _build_tile(nc, tc, T):
    es = ExitStack()

    def pool(name, bufs, space="SBUF"):
        return es.enter_context(tc.tile_pool(name=name, bufs=bufs, space=space))

    singles = pool("singles", 1)
    dram = pool("dram", 1, "DRAM")
    wstream = pool("wstream", 2)    # streamed single-use weights
    enc_pool = pool("enc", 2)
    ctx_pool = pool("ctxp", 2)
    tanh_pool = pool("tanh", 3)
    mm_ps = pool("mm_ps", 4, "PSUM")
    aux_ps = pool("aux_ps", 2, "PSUM")
    small = pool("small", 1)
    logits_pool = pool("logits", 1)
    w2_pool = pool("w2", 2)
    stat = pool("stat", 2)
    opool = pool("out", 2)

    rg = [list(range(NC))]

    def mm_psum():
        return mm_ps.tile([128, 512], F32, tag="mm", name="mmps")

    def aux_psum():
        return aux_ps.tile([128, 512], F32, tag="aux", name="auxps")

    # ---- collective warmup: tiny AllGather, no downstream consumers
    warm_src = dram.tile([1, 64], F32)
    warm_dst = dram.tile([NC, 64], F32)
    s_w = singles.tile([1, 64], F32, tag="warm")
    nc.vector.memset(s_w[:], 0.0)
    nc.gpsimd.dma_start(warm_src[:], s_w[:])
    nc.gpsimd.collective_compute(
        "AllGather", ALU.bypass, replica_groups=rg,
        ins=[warm_src.opt()], outs=[warm_dst.opt()],
    )

    # ---- resident constants in SBUF
    def load_sb(pool_, ap_dram, ktiles, m, dt=BF16, name="w"):
        t = pool_.tile([128, ktiles, m], dt, tag=name, name=name)
        nc.sync.dma_start(t[:], ap_dram.rearrange("(ko p) m -> p ko m", p=128))
        return t

    U_sb = load_sb(singles, T["Uw"], KH, H, name="U_sb")

    hidT_sb = singles.tile([128, KH, NB], BF16)
    nc.sync.dma_start(hidT_sb[:], T["hidT"].rearrange("(ko p) n -> p ko n", p=128))
    hid32_sb = singles.tile([NB, H], F32)
    nc.sync.dma_start(hid32_sb[:], T["hid32"][:])
    vT_sb = singles.tile([128, KH], BF16)
    nc.sync.dma_start(vT_sb[:], T["vT"][:])
    btanh_sb = singles.tile([128, KH], F32)
    nc.sync.dma_start(btanh_sb[:], T["btanh"][:])
    o1bT_sb = singles.tile([128, KH], F32)
    nc.sync.dma_start(o1bT_sb[:], T["o1bT"][:])
    combbT_sb = singles.tile([128, KE], F32)
    nc.sync.dma_start(combbT_sb[:], T["combbT"][:])
    gateb_sb = singles.tile([NB, 2 * H], BF16)
    nc.sync.dma_start(gateb_sb[:], T["gateb"].to_broadcast((NB, 2 * H)))
    candb_sb = singles.tile([NB, H], BF16)
    nc.sync.dma_start(candb_sb[:], T["candb"].to_broadcast((NB, H)))
    hhcb_sb = singles.tile([NB, H], BF16)
    nc.sync.dma_start(hhcb_sb[:], T["hhcb"].to_broadcast((NB, H)))
    lng_sb = singles.tile([NB, H], BF16)
    nc.sync.dma_start(lng_sb[:], T["lng"].to_broadcast((NB, H)))
    lnb_sb = singles.tile([NB, H], BF16)
    nc.sync.dma_start(lnb_sb[:], T["lnb"].to_broadcast((NB, H)))
    w2b_sb = singles.tile([128, VS], BF16)
    nc.sync.dma_start(w2b_sb[:], T["w2b"].to_broadcast((128, VS)))
    ident_bf = singles.tile([128, 128], BF16)
    make_identity(nc, ident_bf[:])
    eps_sb = singles.tile([NB, 1], F32)
    nc.vector.memset(eps_sb[:], EPS)
    sel_sb = singles.tile([128, NB * S], BF16)
    nc.sync.dma_start(sel_sb[:], T["sel"][:])

    # ---- Wh = hidden @ W_w (natural [n, h], bf16, zero-padded to 128 rows)
    Wh_sb = singles.tile([128, H], BF16)
    nc.vector.memset(Wh_sb[:], 0.0)
    Wv = T["Ww"].rearrange("(ko p) m -> p ko m", p=128)
    for nch in range(2):
        sl = slice(512 * nch, 512 * (nch + 1))
        W_c = wstream.tile([128, KH, 512], BF16, tag="wk8", name="W_c")
        nc.sync.dma_start(W_c[:], Wv[:, :, sl])
        ps = mm_psum()
        for k in range(KH):
            nc.tensor.matmul(ps[:NB, :], hidT_sb[:, k, :], W_c[:, k, :],
                             start=(k == 0), stop=(k == KH - 1))
        nc.vector.tensor_copy(Wh_sb[:NB, sl], ps[:NB, :])

    # ---- attention scores: score = tanh(enc@U + Wh + (U_b+W_b)) @ v
    # free layout: (n, s) flattened n-major, chunks of 512 = 4 n x 128 s
    score_dram = dram.tile([NB, S], F32)
    for c in range(SC):
        enc_c = enc_pool.tile([128, KH, 512], BF16, tag="enc_c", name="enc_c")
        nc.sync.dma_start(
            enc_c[:],
            T["encT"][:, 4 * c:4 * (c + 1), :].rearrange(
                "(ko p) n s -> p ko (n s)", p=128),
        )
        sc_ps = aux_psum()
        for m in range(KH):
            ps = mm_psum()
            for k in range(KH):
                nc.tensor.matmul(
                    ps[:], U_sb[:, k, 128 * m:128 * (m + 1)], enc_c[:, k, :],
                    start=(k == 0), stop=False,
                )
            # += Wh[n] via selector matmul (K = n)
            nc.tensor.matmul(
                ps[:], Wh_sb[:, 128 * m:128 * (m + 1)],
                sel_sb[:, 512 * c:512 * (c + 1)],
                start=False, stop=True,
            )
            th = tanh_pool.tile([128, 512], BF16, tag="th", name="th")
            nc.scalar.activation(th[:], ps[:], AF.Tanh, bias=btanh_sb[:, m:m + 1])
            nc.tensor.matmul(
                sc_ps[:1, :], vT_sb[:, m:m + 1], th[:],
                start=(m == 0), stop=(m == KH - 1),
            )
        # score chunk [1, 512] -> scT rows 4c..4c+4 (partition scatter)
        sc_sb = stat.tile([1, 512], F32, tag="sc_sb", name="sc_sb")
        nc.vector.tensor_copy(sc_sb[:], sc_ps[:1, :])
        nc.sync.dma_start(
            score_dram.rearrange("n s -> (n s)")[None, 512 * c:512 * (c + 1)],
            sc_sb[:],
        )

    # ---- softmax over s (per n)
    scT = small.tile([NB, S], F32, tag="scT")
    nc.sync.dma_start(scT[:], score_dram[:])
    nmax = stat.tile([NB, 1], F32, tag="nmax", name="nmax")
    nc.vector.tensor_reduce(nmax[:], scT[:], axis=AX.X, op=ALU.max, negate=True)
    esc = small.tile([NB, S], F32, tag="esc")
    sume = stat.tile([NB, 1], F32, tag="sume", name="sume")
    nc.scalar.activation(esc[:], scT[:], AF.Exp, bias=nmax[:], accum_out=sume[:])
    rsum = stat.tile([NB, 1], F32, tag="rsum", name="rsum")
    nc.vector.reciprocal(rsum[:], sume[:])
    attnT = small.tile([NB, S], F32, tag="attnT")
    nc.vector.tensor_scalar_mul(attnT[:], esc[:], rsum[:])
    # attn out + n-major scratch for context
    attn_dram = dram.tile([NB, S], F32)
    nc.sync.dma_start(attn_dram[:], attnT[:])
    nc.sync.dma_start(T["out_attn"][:], attn_dram.rearrange("n s -> s n"))
    attn_sn = small.tile([128, S * NB], BF16, tag="attn_sn")
    nc.gpsimd.dma_start(
        attn_sn[:],
        attn_dram.rearrange("n s -> (n s)")[None, :].to_broadcast((128, S * NB)),
    )

    # ---- context^T[h, n] = sum_s attn[n,s] * encT[h, n, s]  (on DVE)
    ctxT32 = singles.tile([128, KH, NB], F32)
    encT_kv = T["encT"].rearrange("(ko p) n s -> p ko n s", p=128)
    for k in range(KH):
        for nb4 in range(4):
            enc_k = ctx_pool.tile([128, 8 * S], BF16, tag="enc_k", name="enc_k")
            nc.sync.dma_start(
                enc_k[:],
                encT_kv[:, k, 8 * nb4:8 * (nb4 + 1), :].rearrange(
                    "p n s -> p (n s)"),
            )
            wgt = ctx_pool.tile([128, 8 * S], BF16, tag="wgt", name="wgt")
            nc.vector.tensor_tensor(
                wgt[:], enc_k[:],
                attn_sn[:, 1024 * nb4:1024 * (nb4 + 1)],
                ALU.mult,
            )
            nc.vector.tensor_reduce(
                ctxT32[:, k, 8 * nb4:8 * (nb4 + 1)],
                wgt.rearrange("p (n s) -> p n s", n=8),
                axis=AX.X, op=ALU.add,
            )

    # ---- comb: gT = relu(combw^T @ [x; ctx]^T + comb_b)  [E(part,tiles), n]
    combv = T["combw"].rearrange("(ko p) m -> p ko m", p=128)
    xctxT = singles.tile([128, KH + KE, NB], BF16)
    nc.sync.dma_start(
        xctxT[:, :KE, :], T["xT"].rearrange("(ko p) n -> p ko n", p=128)
    )
    nc.vector.tensor_copy(xctxT[:, KE:, :], ctxT32[:])
    gT_sb = singles.tile([128, KE, NB], BF16)
    for m in range(KE):
        comb_m = wstream.tile([128, KH + KE, 128], BF16, tag="wlhs12",
                              name="comb_m")
        nc.sync.dma_start(comb_m[:], combv[:, :, 128 * m:128 * (m + 1)])
        ps = aux_psum()
        for k in range(KH + KE):
            nc.tensor.matmul(
                ps[:, :NB], comb_m[:, k, :], xctxT[:, k, :],
                start=(k == 0), stop=(k == KH + KE - 1),
            )
        nc.scalar.activation(gT_sb[:, m, :], ps[:, :NB], AF.Relu,
                             bias=combbT_sb[:, m:m + 1])

    # ---- GRU gates (natural layout [n, f]): sigma(g@ihw + hid@hhw + b)
    ihv = T["ihw"].rearrange("(ko p) m -> p ko m", p=128)
    hhv = T["hhw"].rearrange("(ko p) m -> p ko m", p=128)
    gates = small.tile([NB, 2 * H], F32, tag="gates")
    for nch in range(4):
        sl = slice(512 * nch, 512 * (nch + 1))
        wih = wstream.tile([128, KE, 512], BF16, tag="wk4", name="wih")
        nc.sync.dma_start(wih[:], ihv[:, :, sl])
        whh = wstream.tile([128, KH, 512], BF16, tag="wk8", name="whh")
        nc.sync.dma_start(whh[:], hhv[:, :, sl])
        ps = mm_psum()
        for k in range(KE):
            nc.tensor.matmul(ps[:NB, :], gT_sb[:, k, :], wih[:, k, :],
                             start=(k == 0), stop=False)
        for k in range(KH):
            nc.tensor.matmul(ps[:NB, :], hidT_sb[:, k, :], whh[:, k, :],
                             start=False, stop=(k == KH - 1))
        nc.vector.tensor_tensor(
            gates[:, sl], ps[:NB, :], gateb_sb[:, sl], ALU.add,
        )
    # sigmoid(x) = 0.5 * (1 + tanh(x/2))
    nc.scalar.activation(gates[:], gates[:], AF.Tanh, scale=0.5)
    nc.vector.tensor_scalar(gates[:], gates[:], 0.5, 0.5, ALU.mult, ALU.add)
    z_sl, r_sl = slice(0, H), slice(H, 2 * H)

    # ---- candidate: tanh(g@candw + cand_b + r * (hid@hhcw + hhc_b))
    hhcv = T["hhcw"].rearrange("(ko p) m -> p ko m", p=128)
    candv = T["candw"].rearrange("(ko p) m -> p ko m", p=128)
    hhc = small.tile([NB, H], F32, tag="hhc")
    for nch in range(2):
        sl = slice(512 * nch, 512 * (nch + 1))
        whhc = wstream.tile([128, KH, 512], BF16, tag="wk8", name="whhc")
        nc.sync.dma_start(whhc[:], hhcv[:, :, sl])
        ps = mm_psum()
        for k in range(KH):
            nc.tensor.matmul(ps[:NB, :], hidT_sb[:, k, :], whhc[:, k, :],
                             start=(k == 0), stop=(k == KH - 1))
        nc.vector.tensor_tensor(
            hhc[:, sl], ps[:NB, :], hhcb_sb[:, sl], ALU.add,
        )
    nc.vector.tensor_tensor(hhc[:], hhc[:], gates[:, r_sl], ALU.mult)
    cand = small.tile([NB, H], F32, tag="cand")
    for nch in range(2):
        sl = slice(512 * nch, 512 * (nch + 1))
        wcand = wstream.tile([128, KE, 512], BF16, tag="wk4", name="wcand")
        nc.sync.dma_start(wcand[:], candv[:, :, sl])
        ps = mm_psum()
        for k in range(KE):
            nc.tensor.matmul(ps[:NB, :], gT_sb[:, k, :], wcand[:, k, :],
                             start=(k == 0), stop=(k == KE - 1))
        nc.vector.tensor_tensor(cand[:, sl], ps[:NB, :], hhc[:, sl], ALU.add)
    nc.vector.tensor_tensor(cand[:], cand[:], candb_sb[:], ALU.add)
    nc.scalar.activation(cand[:], cand[:], AF.Tanh)

    # ---- h = hidden + z*(cand - hidden); LayerNorm
    hpre = small.tile([NB, H], F32, tag="hpre")
    nc.vector.tensor_tensor(hpre[:], cand[:], hid32_sb[:], ALU.subtract)
    nc.vector.tensor_tensor(hpre[:], hpre[:], gates[:, z_sl], ALU.mult)
    nc.vector.tensor_tensor(hpre[:], hpre[:], hid32_sb[:], ALU.add)
    mv = stat.tile([NB, 2, 6], F32, tag="mv", name="mv")
    nc.vector.bn_stats(mv[:, 0, :], hpre[:, 0:512])
    nc.vector.bn_stats(mv[:, 1, :], hpre[:, 512:1024])
    mu_var = stat.tile([NB, 2], F32, tag="mu_var", name="mu_var")
    nc.vector.bn_aggr(mu_var[:], mv[:])
    # rstd = exp(-0.5 * ln(var + eps))
    lnv = stat.tile([NB, 1], F32, tag="lnv", name="lnv")
    nc.scalar.activation(lnv[:], mu_var[:, 1:2], AF.Ln, bias=eps_sb[:])
    rstd = stat.tile([NB, 1], F32, tag="rstd", name="rstd")
    nc.scalar.activation(rstd[:], lnv[:], AF.Exp, scale=-0.5)
    nc.vector.tensor_scalar(hpre[:], hpre[:], mu_var[:, 0:1], rstd[:],
                            ALU.subtract, ALU.mult)
    nc.vector.tensor_tensor(hpre[:], hpre[:], lng_sb[:], ALU.mult)
    nc.vector.tensor_tensor(hpre[:], hpre[:], lnb_sb[:], ALU.add)
    nc.sync.dma_start(T["out_h"][:], hpre[:])

    # ---- transpose h -> hT (bf16), gather across cores
    hbf = small.tile([NB, H], BF16, tag="hbf")
    nc.vector.tensor_copy(hbf[:], hpre[:])
    hT_sb = singles.tile([128, KH, NB], BF16)
    for k in range(KH):
        tps = aux_ps.tile([128, NB], BF16, tag="trh", name="trh")
        nc.tensor.transpose(tps[:], hbf[:, 128 * k:128 * (k + 1)],
                            ident_bf[:NB, :NB])
        nc.vector.tensor_copy(hT_sb[:, k, :], tps[:])
    hT_src = dram.tile([H, NB], BF16)
    nc.gpsimd.dma_start(hT_src.rearrange("(ko p) n -> p ko n", p=128), hT_sb[:])
    hT_all = dram.tile([NC, H, NB], BF16)
    nc.gpsimd.collective_compute(
        "AllGather", ALU.bypass, replica_groups=rg,
        ins=[hT_src.opt()], outs=[hT_all.opt()],
    )

    # ---- out1: o1T = relu(o1w^T @ hT_full + o1b)   [h(part,tiles), n_full]
    o1v = T["o1w"].rearrange("(ko p) m -> p ko m", p=128)
    o1T_sb = singles.tile([128, KH, N], BF16)
    hTf_sb = singles.tile([128, KH, N], BF16)
    for k in range(KH):
        nc.sync.dma_start(
            hTf_sb[:, k, :].rearrange("p (r n) -> p r n", r=NC),
            hT_all[:, 128 * k:128 * (k + 1), :].rearrange("r p n -> p r n"),
        )
    for m in range(KH):
        o1_m = wstream.tile([128, KH, 128], BF16, tag="wlhs8", name="o1_m")
        nc.sync.dma_start(o1_m[:], o1v[:, :, 128 * m:128 * (m + 1)])
        ps = mm_psum()
        for k in range(KH):
            nc.tensor.matmul(ps[:, :N], o1_m[:, k, :],
                             hTf_sb[:, k, :],
                             start=(k == 0), stop=(k == KH - 1))
        nc.scalar.activation(o1T_sb[:, m, :], ps[:, :N], AF.Relu,
                             bias=o1bT_sb[:, m:m + 1])

    # ---- out2 (vocab shard): logits[n, v] resident in SBUF bf16
    w2v = T["w2"].rearrange("(ko p) v -> p ko v", p=128)
    lg0 = logits_pool.tile([128, VS], BF16, tag="lg0")
    lg1 = logits_pool.tile([128, VS], BF16, tag="lg1")
    lg = [lg0, lg1]
    for vc in range(2 * VCH):
        w2_c = w2_pool.tile([128, KH, 256], BF16, tag="w2c", name="w2c")
        nc.sync.dma_start(w2_c[:], w2v[:, :, 256 * vc:256 * (vc + 1)])
        for mt in range(2):
            ps = mm_psum()
            for k in range(KH):
                nc.tensor.matmul(
                    ps[:, :256], o1T_sb[:, k, 128 * mt:128 * (mt + 1)],
                    w2_c[:, k, :],
                    start=(k == 0), stop=(k == KH - 1),
                )
            nc.vector.tensor_tensor(
                lg[mt][:, 256 * vc:256 * (vc + 1)], ps[:, :256],
                w2b_sb[:, 256 * vc:256 * (vc + 1)], ALU.add,
            )

    # ---- local log-softmax stats -> AllGather -> lse -> normalize
    st_src = dram.tile([4, 128], F32)
    for mt in range(2):
        rm = stat.tile([128, 1], F32, tag=f"rm{mt}", name=f"rm{mt}")
        nc.vector.tensor_reduce(rm[:], lg[mt][:], axis=AX.X, op=ALU.max)
        nrm = stat.tile([128, 1], F32, tag=f"nrm{mt}", name=f"nrm{mt}")
        nc.vector.tensor_scalar_mul(nrm[:], rm[:], -1.0)
        parts = stat.tile([128, VCH], F32, tag=f"parts{mt}", name=f"parts{mt}")
        for vc in range(VCH):
            etmp = opool.tile([128, 512], BF16, tag="etmp", name="etmp")
            nc.scalar.activation(
                etmp[:], lg[mt][:, 512 * vc:512 * (vc + 1)], AF.Exp,
                bias=nrm[:], accum_out=parts[:, vc:vc + 1],
            )
        se = stat.tile([128, 1], F32, tag=f"se{mt}", name=f"se{mt}")
        nc.vector.tensor_reduce(se[:], parts[:], axis=AX.X, op=ALU.add)
        nc.sync.dma_start(st_src[2 * mt, :, None], rm[:])
        nc.sync.dma_start(st_src[2 * mt + 1, :, None], se[:])

    st_all = dram.tile([NC, 4, 128], F32)
    nc.gpsimd.collective_compute(
        "AllGather", ALU.bypass, replica_groups=rg,
        ins=[st_src.opt()], outs=[st_all.opt()],
    )

    # ---- combine stats: lse[n] = gmax + ln(sum_r exp(max_r - gmax) * S_r)
    for mt in range(2):
        gm_all = stat.tile([128, NC], F32, tag=f"gma{mt}", name=f"gma{mt}")
        nc.sync.dma_start(gm_all[:], st_all[:, 2 * mt, :].rearrange("r p -> p r"))
        se_all = stat.tile([128, NC], F32, tag=f"sea{mt}", name=f"sea{mt}")
        nc.sync.dma_start(se_all[:],
                          st_all[:, 2 * mt + 1, :].rearrange("r p -> p r"))
        ngmax = stat.tile([128, 1], F32, tag=f"ngm{mt}", name=f"ngm{mt}")
        nc.vector.tensor_reduce(ngmax[:], gm_all[:], axis=AX.X, op=ALU.max,
                                negate=True)
        ed = stat.tile([128, NC], F32, tag=f"ed{mt}", name=f"ed{mt}")
        nc.scalar.activation(ed[:], gm_all[:], AF.Exp, bias=ngmax[:])
        nc.vector.tensor_tensor(ed[:], ed[:], se_all[:], ALU.mult)
        ssum = stat.tile([128, 1], F32, tag=f"ssum{mt}", name=f"ssum{mt}")
        nc.vector.tensor_reduce(ssum[:], ed[:], axis=AX.X, op=ALU.add)
        lns = stat.tile([128, 1], F32, tag=f"lns{mt}", name=f"lns{mt}")
        nc.scalar.activation(lns[:], ssum[:], AF.Ln)
        nlse = stat.tile([128, 1], F32, tag=f"nlse{mt}", name=f"nlse{mt}")
        nc.vector.tensor_tensor(nlse[:], ngmax[:], lns[:], ALU.subtract)
        for vc in range(VCH):
            oc = opool.tile([128, 512], F32, tag="oc", name="oc")
            nc.vector.tensor_scalar_add(oc[:], lg[mt][:, 512 * vc:512 * (vc + 1)],
                                        nlse[:])
            nc.sync.dma_start(
                T["out_lg"][128 * mt:128 * (mt + 1), 512 * vc:512 * (vc + 1)],
                oc[:],
            )

    es.close()


_CACHE = {}


def _get_nc():
    if "nc" not in _CACHE:
        _CACHE["nc"] = build_program()
    return _CACHE["nc"]


def _tile_p(v):
    return np.ascontiguousarray(np.asarray(v, dtype=np.float32).reshape(-1, 128).T)


def prep_inputs(inputs):
    """Shard + cast the full inputs into 8 per-core input maps."""
    ids = np.asarray(inputs["input_ids"])
    hidden = np.asarray(inputs["hidden"], dtype=np.float32)
    enc = np.asarray(inputs["encoder_outputs"], dtype=np.float32)
    emb = np.asarray(inputs["emb"], dtype=np.float32)
    x = emb[ids]  # [N, E] embedding row gather (pure indexing)

    f32 = lambda a: np.ascontiguousarray(a, dtype=np.float32)
    b16 = lambda a: np.ascontiguousarray(np.asarray(a, dtype=np.float32).astype(bf16))

    btanh_t = _tile_p(np.asarray(inputs["W_b"]) + np.asarray(inputs["U_b"]))
    o1bT_t = _tile_p(inputs["out1_b"])
    combbT_t = _tile_p(inputs["comb_b"])
    vT_t = _tile_p(np.asarray(inputs["v_w"], dtype=np.float32)[:, 0]).astype(bf16)
    gateb = b16(np.asarray(inputs["ih_b"]) + np.asarray(inputs["hh_b"]))[None, :]
    candb = b16(inputs["cand_b"])[None, :]
    hhcb = b16(inputs["hhc_b"])[None, :]
    lng = b16(inputs["ln_g"])[None, :]
    lnb = b16(inputs["ln_b"])[None, :]

    w2_full = np.asarray(inputs["out2_w"], dtype=np.float32)
    w2b_full = np.asarray(inputs["out2_b"], dtype=np.float32)

    sel = np.zeros((128, NB * S), dtype=np.float32)
    sel[:NB] = np.repeat(np.eye(NB, dtype=np.float32), S, axis=1)
    sel = np.ascontiguousarray(sel.astype(bf16))
    shared = {
        "sel": sel,
        "Uw": b16(inputs["U_w"]), "Ww": b16(inputs["W_w"]),
        "o1w": b16(inputs["out1_w"]), "combw": b16(inputs["comb_w"]),
        "ihw": b16(inputs["ih_w"]), "hhw": b16(inputs["hh_w"]),
        "candw": b16(inputs["cand_w"]), "hhcw": b16(inputs["hhc_w"]),
        "vT": np.ascontiguousarray(vT_t), "btanh": btanh_t,
        "o1bT": o1bT_t, "combbT": combbT_t, "gateb": gateb,
        "candb": candb, "hhcb": hhcb, "lng": lng, "lnb": lnb,
    }

    in_maps = []
    for c in range(NC):
        nb = slice(NB * c, NB * (c + 1))
        vs, ve = VS * c, min(VS * (c + 1), V)
        w2c = np.zeros((H, VS), dtype=bf16)
        w2c[:, :ve - vs] = w2_full[:, vs:ve].astype(bf16)
        w2bc = np.full((1, VS), NEG_BIG, dtype=np.float32)
        w2bc[0, :ve - vs] = w2b_full[vs:ve]
        w2bc = w2bc.astype(bf16)
        m = dict(shared)
        m["encT"] = b16(enc[:, nb, :].transpose(2, 1, 0))
        m["hidT"] = b16(hidden[nb].T)
        m["hid32"] = f32(hidden[nb])
        m["xT"] = b16(x[nb].T)
        m["w2"] = w2c
        m["w2b"] = w2bc
        in_maps.append(m)
    return in_maps


def run(inputs, trace=False):
    nc = _get_nc()
    in_maps = prep_inputs(inputs)
    res = bass_utils.run_bass_kernel_spmd(
        nc, in_maps, core_ids=list(range(NC)), trace=trace
    )
    out = np.empty((N, V), dtype=np.float32)
    h = np.empty((N, H), dtype=np.float32)
    attn = np.empty((S, N, 1), dtype=np.float32)
    for c in range(NC):
        r = res.results[c]
        vs, ve = VS * c, min(VS * (c + 1), V)
        out[:, vs:ve] = r["out_lg"][:, :ve - vs]
        h[NB * c:NB * (c + 1)] = r["out_h"]
        attn[:, NB * c:NB * (c + 1), 0] = r["out_attn"]
    return (out, h, attn), res


def kernel(**inputs):
    (out, h, attn), _ = run(inputs)
    return out, h, attn


# revision 20
# speedup vs baseline: 1.2020x; 1.2020x over previous
"""Trainium2 Bass kernel for nn_DecoderRNN (Bahdanau attention + GRU cell +
LayerNorm + vocab projection + log-softmax), SPMD over 8 NeuronCores.

Sharding: batch (N=256 -> 32/core) for attention/GRU; vocab (V=50257 ->
6656/core padded) for the H->V projection; one AllGather of the hidden state
between the phases and one tiny AllGather for the log-softmax statistics.
"""
from contextlib import ExitStack

import numpy as np
import ml_dtypes

import concourse.bass as bass
import concourse.tile as tile
from concourse import bacc, mybir
from concourse import bass_utils
from concourse.masks import make_identity

BF16 = mybir.dt.bfloat16
F32 = mybir.dt.float32
AF = mybir.ActivationFunctionType
ALU = mybir.AluOpType
AX = mybir.AxisListType

# Model dims (hardcoded per problem spec)
V, E, H, N, S = 50257, 512, 1024, 256, 128
EPS = 1e-5
NC = 8            # cores
NB = N // NC      # batch rows per core = 32
VS = 6656         # vocab shard (13 * 512), 8*6656 = 53248 >= V
VCH = VS // 512   # 13 v-chunks
KH = H // 128     # 8 k-tiles over H
KE = E // 128     # 4 k-tiles over E
SC = 8            # score chunks: (s,n) = 4096 -> 8 chunks of 512 (16 s each)
NEG_BIG = -1e30

bf16 = ml_dtypes.bfloat16


def build_program():
    nc = bacc.Bacc("TRN2", target_bir_lowering=False, debug=False, num_devices=NC)

    def din(name, shape, dt=BF16):
        return nc.dram_tensor(name, shape, dt, kind="ExternalInput").ap()

    T = {}
    # --- external inputs (per core) ---
    T["encT"] = din("encT", (H, NB, S))              # enc transposed, bf16
    T["hidT"] = din("hidT", (H, NB))                 # hidden transposed, bf16
    T["hid32"] = din("hid32", (NB, H), F32)          # hidden natural, f32
    T["xT"] = din("xT", (E, NB))                     # emb rows transposed, bf16
    T["Uw"] = din("Uw", (H, H))
    T["Ww"] = din("Ww", (H, H))
    T["o1w"] = din("o1w", (H, H))
    T["combw"] = din("combw", (H + E, E))
    T["ihw"] = din("ihw", (E, 2 * H))
    T["hhw"] = din("hhw", (H, 2 * H))
    T["candw"] = din("candw", (E, H))
    T["hhcw"] = din("hhcw", (H, H))
    T["w2"] = din("w2", (H, VS))
    T["vT"] = din("vT", (128, KH))
    T["btanh"] = din("btanh", (128, KH), F32)        # W_b + U_b, tiled
    T["o1bT"] = din("o1bT", (128, KH), F32)
    T["combbT"] = din("combbT", (128, KE), F32)
    T["gateb"] = din("gateb", (1, 2 * H))       # ih_b + hh_b
    T["candb"] = din("candb", (1, H))
    T["hhcb"] = din("hhcb", (1, H))
    T["lng"] = din("lng", (1, H))
    T["lnb"] = din("lnb", (1, H))
    T["w2b"] = din("w2b", (1, VS))              # padded with -1e30
    T["sel"] = din("sel", (128, NB * S))        # Wh broadcast selector (zero-padded K)

    # --- external outputs (per core) ---
    T["out_lg"] = nc.dram_tensor("out_lg", (N, VS), F32, kind="ExternalOutput").ap()
    T["out_h"] = nc.dram_tensor("out_h", (NB, H), F32, kind="ExternalOutput").ap()
    T["out_attn"] = nc.dram_tensor("out_attn", (S, NB), F32,
                                   kind="ExternalOutput").ap()

    with tile.TileContext(nc) as tc:
        _build_tile(nc, tc, T)
    nc.compile()
    return nc


def _build_tile(nc, tc, T):
    es = ExitStack()

    def pool(name, bufs, space="SBUF"):
        return es.enter_context(tc.tile_pool(name=name, bufs=bufs, space=space))

    singles = pool("singles", 1)
    dram = pool("dram", 1, "DRAM")
    wstream = pool("wstream", 2)    # streamed single-use weights
    enc_pool = pool("enc", 2)
    ctx_pool = pool("ctxp", 2)
    tanh_pool = pool("tanh", 3)
    mm_ps = pool("mm_ps", 4, "PSUM")
    aux_ps = pool("aux_ps", 2, "PSUM")
    small = pool("small", 1)
    logits_pool = pool("logits", 1)
    w2_pool = pool("w2", 2)
    stat = pool("stat", 2)
    opool = pool("out", 2)

    rg = [list(range(NC))]

    def mm_psum():
        return mm_ps.tile([128, 512], F32, tag="mm", name="mmps")

    def aux_psum():
        return aux_ps.tile([128, 512], F32, tag="aux", name="auxps")

    # ---- collective warmup: tiny AllGather, no downstream consumers
    warm_src = dram.tile([1, 64], F32)
    warm_dst = dram.tile([NC, 64], F32)
    s_w = singles.tile([1, 64], F32, tag="warm")
    nc.vector.memset(s_w[:], 0.0)
    nc.gpsimd.dma_start(warm_src[:], s_w[:])
    nc.gpsimd.collective_compute(
        "AllGather", ALU.bypass, replica_groups=rg,
        ins=[warm_src.opt()], outs=[warm_dst.opt()],
    )

    # ---- resident constants in SBUF
    def load_sb(pool_, ap_dram, ktiles, m, dt=BF16, name="w"):
        t = pool_.tile([128, ktiles, m], dt, tag=name, name=name)
        nc.sync.dma_start(t[:], ap_dram.rearrange("(ko p) m -> p ko m", p=128))
        return t

    U_sb = load_sb(singles, T["Uw"], KH, H, name="U_sb")

    hidT_sb = singles.tile([128, KH, NB], BF16)
    nc.sync.dma_start(hidT_sb[:], T["hidT"].rearrange("(ko p) n -> p ko n", p=128))
    hid32_sb = singles.tile([NB, H], F32)
    nc.sync.dma_start(hid32_sb[:], T["hid32"][:])
    vT_sb = singles.tile([128, KH], BF16)
    nc.sync.dma_start(vT_sb[:], T["vT"][:])
    btanh_sb = singles.tile([128, KH], F32)
    nc.sync.dma_start(btanh_sb[:], T["btanh"][:])
    o1bT_sb = singles.tile([128, KH], F32)
    nc.sync.dma_start(o1bT_sb[:], T["o1bT"][:])
    combbT_sb = singles.tile([128, KE], F32)
    nc.sync.dma_start(combbT_sb[:], T["combbT"][:])
    gateb_sb = singles.tile([NB, 2 * H], BF16)
    nc.sync.dma_start(gateb_sb[:], T["gateb"].to_broadcast((NB, 2 * H)))
    candb_sb = singles.tile([NB, H], BF16)
    nc.sync.dma_start(candb_sb[:], T["candb"].to_broadcast((NB, H)))
    hhcb_sb = singles.tile([NB, H], BF16)
    nc.sync.dma_start(hhcb_sb[:], T["hhcb"].to_broadcast((NB, H)))
    lng_sb = singles.tile([NB, H], BF16)
    nc.sync.dma_start(lng_sb[:], T["lng"].to_broadcast((NB, H)))
    lnb_sb = singles.tile([NB, H], BF16)
    nc.sync.dma_start(lnb_sb[:], T["lnb"].to_broadcast((NB, H)))
    w2b_sb = singles.tile([128, VS], BF16)
    nc.sync.dma_start(w2b_sb[:], T["w2b"].to_broadcast((128, VS)))
    ident_bf = singles.tile([128, 128], BF16)
    make_identity(nc, ident_bf[:])
    eps_sb = singles.tile([NB, 1], F32)
    nc.vector.memset(eps_sb[:], EPS)
    sel_sb = singles.tile([128, NB * S], BF16)
    nc.sync.dma_start(sel_sb[:], T["sel"][:])

    # ---- Wh = hidden @ W_w (natural [n, h], bf16, zero-padded to 128 rows)
    Wh_sb = singles.tile([128, H], BF16)
    nc.vector.memset(Wh_sb[:], 0.0)
    Wv = T["Ww"].rearrange("(ko p) m -> p ko m", p=128)
    for nch in range(2):
        sl = slice(512 * nch, 512 * (nch + 1))
        W_c = wstream.tile([128, KH, 512], BF16, tag="wk8", name="W_c")
        nc.sync.dma_start(W_c[:], Wv[:, :, sl])
        ps = mm_psum()
        for k in range(KH):
            nc.tensor.matmul(ps[:NB, :], hidT_sb[:, k, :], W_c[:, k, :],
                             start=(k == 0), stop=(k == KH - 1))
        nc.vector.tensor_copy(Wh_sb[:NB, sl], ps[:NB, :])

    # ---- attention: per 4-row n-chunk: scores -> softmax -> context,
    # all pipelined under the PE matmul stream.
    score_dram = dram.tile([NB, S], F32)
    attn_dram = dram.tile([NB, S], F32)
    ctxT32 = singles.tile([128, KH, NB], F32)
    identf = singles.tile([NB, NB], F32)
    make_identity(nc, identf[:])
    for c in range(SC):
        enc_c = enc_pool.tile([128, KH, 512], BF16, tag="enc_c", name="enc_c")
        nc.sync.dma_start(
            enc_c[:],
            T["encT"][:, 4 * c:4 * (c + 1), :].rearrange(
                "(ko p) n s -> p ko (n s)", p=128),
        )
        ths = []
        for m in range(KH):
            ps = mm_psum()
            for k in range(KH):
                nc.tensor.matmul(
                    ps[:], U_sb[:, k, 128 * m:128 * (m + 1)], enc_c[:, k, :],
                    start=(k == 0), stop=False,
                )
            # += Wh[n] via selector matmul (K rows 32..127 are zero)
            nc.tensor.matmul(
                ps[:], Wh_sb[:, 128 * m:128 * (m + 1)],
                sel_sb[:, 512 * c:512 * (c + 1)],
                start=False, stop=True,
            )
            th = tanh_pool.tile([128, 512], BF16, tag="th", name="th")
            nc.scalar.activation(th[:], ps[:], AF.Tanh, bias=btanh_sb[:, m:m + 1])
            ths.append(th)
        sc_ps = aux_psum()
        for m in range(KH):
            nc.tensor.matmul(
                sc_ps[:1, :], vT_sb[:, m:m + 1], ths[m][:],
                start=(m == 0), stop=(m == KH - 1),
            )
        # score chunk [1, 512] -> DRAM -> [4, 128] rows; softmax over s
        sc_sb = stat.tile([1, 512], F32, tag="sc_sb", name="sc_sb")
        nc.vector.tensor_copy(sc_sb[:], sc_ps[:1, :])
        nc.sync.dma_start(
            score_dram.rearrange("n s -> (n s)")[None, 512 * c:512 * (c + 1)],
            sc_sb[:],
        )
        sc4 = stat.tile([4, S], F32, tag="sc4", name="sc4")
        nc.sync.dma_start(sc4[:], score_dram[4 * c:4 * (c + 1), :])
        nmax4 = stat.tile([4, 1], F32, tag="nmax4", name="nmax4")
        nc.vector.tensor_reduce(nmax4[:], sc4[:], axis=AX.X, op=ALU.max,
                                negate=True)
        e4 = stat.tile([4, S], F32, tag="e4", name="e4")
        sum4 = stat.tile([4, 1], F32, tag="sum4", name="sum4")
        nc.scalar.activation(e4[:], sc4[:], AF.Exp, bias=nmax4[:],
                             accum_out=sum4[:])
        r4 = stat.tile([4, 1], F32, tag="r4", name="r4")
        nc.vector.reciprocal(r4[:], sum4[:])
        attn4 = stat.tile([4, S], F32, tag="attn4", name="attn4")
        nc.vector.tensor_scalar_mul(attn4[:], e4[:], r4[:])
        nc.sync.dma_start(attn_dram[4 * c:4 * (c + 1), :], attn4[:])
        attn_bc = ctx_pool.tile([128, 512], BF16, tag="abc", name="abc")
        nc.gpsimd.dma_start(
            attn_bc[:],
            attn_dram.rearrange("n s -> (n s)")[
                None, 512 * c:512 * (c + 1)].to_broadcast((128, 512)),
        )
        # context partial: ctxT[:, k, 4c:4c+4] = sum_s enc_c * attn
        for k in range(KH):
            wgt = ctx_pool.tile([128, 512], BF16, tag="wgt", name="wgt")
            nc.vector.tensor_tensor(wgt[:], enc_c[:, k, :], attn_bc[:], ALU.mult)
            nc.vector.tensor_reduce(
                ctxT32[:, k, 4 * c:4 * (c + 1)],
                wgt.rearrange("p (n s) -> p n s", n=4),
                axis=AX.X, op=ALU.add,
            )

    # ---- attn output: [NB, S] -> transpose on PE -> [S, NB] contiguous DMA
    attnT = small.tile([NB, S], F32, tag="attnT")
    nc.sync.dma_start(attnT[:], attn_dram[:])
    atr_ps = aux_psum()
    nc.tensor.transpose(atr_ps[:S, :NB], attnT[:], identf[:])
    attnS = small.tile([S, NB], F32, tag="attnS")
    nc.vector.tensor_copy(attnS[:], atr_ps[:S, :NB])
    nc.sync.dma_start(T["out_attn"][:], attnS[:])

    # ---- comb: gT = relu(combw^T @ [x; ctx]^T + comb_b)  [E(part,tiles), n]
    combv = T["combw"].rearrange("(ko p) m -> p ko m", p=128)
    xctxT = singles.tile([128, KH + KE, NB], BF16)
    nc.sync.dma_start(
        xctxT[:, :KE, :], T["xT"].rearrange("(ko p) n -> p ko n", p=128)
    )
    nc.vector.tensor_copy(xctxT[:, KE:, :], ctxT32[:])
    gT_sb = singles.tile([128, KE, NB], BF16)
    for m in range(KE):
        comb_m = wstream.tile([128, KH + KE, 128], BF16, tag="wlhs12",
                              name="comb_m")
        nc.sync.dma_start(comb_m[:], combv[:, :, 128 * m:128 * (m + 1)])
        ps = aux_psum()
        for k in range(KH + KE):
            nc.tensor.matmul(
                ps[:, :NB], comb_m[:, k, :], xctxT[:, k, :],
                start=(k == 0), stop=(k == KH + KE - 1),
            )
        nc.scalar.activation(gT_sb[:, m, :], ps[:, :NB], AF.Relu,
                             bias=combbT_sb[:, m:m + 1])

    # ---- GRU gates (natural layout [n, f]): sigma(g@ihw + hid@hhw + b)
    ihv = T["ihw"].rearrange("(ko p) m -> p ko m", p=128)
    hhv = T["hhw"].rearrange("(ko p) m -> p ko m", p=128)
    gates = small.tile([NB, 2 * H], F32, tag="gates")
    for nch in range(4):
        sl = slice(512 * nch, 512 * (nch + 1))
        wih = wstream.tile([128, KE, 512], BF16, tag="wk4", name="wih")
        nc.sync.dma_start(wih[:], ihv[:, :, sl])
        whh = wstream.tile([128, KH, 512], BF16, tag="wk8", name="whh")
        nc.sync.dma_start(whh[:], hhv[:, :, sl])
        ps = mm_psum()
        for k in range(KE):
            nc.tensor.matmul(ps[:NB, :], gT_sb[:, k, :], wih[:, k, :],
                             start=(k == 0), stop=False)
        for k in range(KH):
            nc.tensor.matmul(ps[:NB, :], hidT_sb[:, k, :], whh[:, k, :],
                             start=False, stop=(k == KH - 1))
        nc.vector.tensor_tensor(
            gates[:, sl], ps[:NB, :], gateb_sb[:, sl], ALU.add,
        )
    # sigmoid(x) = 0.5 * (1 + tanh(x/2))
    nc.scalar.activation(gates[:], gates[:], AF.Tanh, scale=0.5)
    nc.vector.tensor_scalar(gates[:], gates[:], 0.5, 0.5, ALU.mult, ALU.add)
    z_sl, r_sl = slice(0, H), slice(H, 2 * H)

    # ---- candidate: tanh(g@candw + cand_b + r * (hid@hhcw + hhc_b))
    hhcv = T["hhcw"].rearrange("(ko p) m -> p ko m", p=128)
    candv = T["candw"].rearrange("(ko p) m -> p ko m", p=128)
    hhc = small.tile([NB, H], F32, tag="hhc")
    for nch in range(2):
        sl = slice(512 * nch, 512 * (nch + 1))
        whhc = wstream.tile([128, KH, 512], BF16, tag="wk8", name="whhc")
        nc.sync.dma_start(whhc[:], hhcv[:, :, sl])
        ps = mm_psum()
        for k in range(KH):
            nc.tensor.matmul(ps[:NB, :], hidT_sb[:, k, :], whhc[:, k, :],
                             start=(k == 0), stop=(k == KH - 1))
        nc.vector.tensor_tensor(
            hhc[:, sl], ps[:NB, :], hhcb_sb[:, sl], ALU.add,
        )
    nc.vector.tensor_tensor(hhc[:], hhc[:], gates[:, r_sl], ALU.mult)
    cand = small.tile([NB, H], F32, tag="cand")
    for nch in range(2):
        sl = slice(512 * nch, 512 * (nch + 1))
        wcand = wstream.tile([128, KE, 512], BF16, tag="wk4", name="wcand")
        nc.sync.dma_start(wcand[:], candv[:, :, sl])
        ps = mm_psum()
        for k in range(KE):
            nc.tensor.matmul(ps[:NB, :], gT_sb[:, k, :], wcand[:, k, :],
                             start=(k == 0), stop=(k == KE - 1))
        nc.vector.tensor_tensor(cand[:, sl], ps[:NB, :], hhc[:, sl], ALU.add)
    nc.vector.tensor_tensor(cand[:], cand[:], candb_sb[:], ALU.add)
    nc.scalar.activation(cand[:], cand[:], AF.Tanh)

    # ---- h = hidden + z*(cand - hidden); LayerNorm
    hpre = small.tile([NB, H], F32, tag="hpre")
    nc.vector.tensor_tensor(hpre[:], cand[:], hid32_sb[:], ALU.subtract)
    nc.vector.tensor_tensor(hpre[:], hpre[:], gates[:, z_sl], ALU.mult)
    nc.vector.tensor_tensor(hpre[:], hpre[:], hid32_sb[:], ALU.add)
    mv = stat.tile([NB, 2, 6], F32, tag="mv", name="mv")
    nc.vector.bn_stats(mv[:, 0, :], hpre[:, 0:512])
    nc.vector.bn_stats(mv[:, 1, :], hpre[:, 512:1024])
    mu_var = stat.tile([NB, 2], F32, tag="mu_var", name="mu_var")
    nc.vector.bn_aggr(mu_var[:], mv[:])
    # rstd = exp(-0.5 * ln(var + eps))
    lnv = stat.tile([NB, 1], F32, tag="lnv", name="lnv")
    nc.scalar.activation(lnv[:], mu_var[:, 1:2], AF.Ln, bias=eps_sb[:])
    rstd = stat.tile([NB, 1], F32, tag="rstd", name="rstd")
    nc.scalar.activation(rstd[:], lnv[:], AF.Exp, scale=-0.5)
    nc.vector.tensor_scalar(hpre[:], hpre[:], mu_var[:, 0:1], rstd[:],
                            ALU.subtract, ALU.mult)
    nc.vector.tensor_tensor(hpre[:], hpre[:], lng_sb[:], ALU.mult)
    nc.vector.tensor_tensor(hpre[:], hpre[:], lnb_sb[:], ALU.add)
    nc.sync.dma_start(T["out_h"][:], hpre[:])

    # ---- transpose h -> hT (bf16), gather across cores
    hbf = small.tile([NB, H], BF16, tag="hbf")
    nc.vector.tensor_copy(hbf[:], hpre[:])
    hT_sb = singles.tile([128, KH, NB], BF16)
    for k in range(KH):
        tps = aux_ps.tile([128, NB], BF16, tag="trh", name="trh")
        nc.tensor.transpose(tps[:], hbf[:, 128 * k:128 * (k + 1)],
                            ident_bf[:NB, :NB])
        nc.vector.tensor_copy(hT_sb[:, k, :], tps[:])
    hT_src = dram.tile([H, NB], BF16)
    nc.gpsimd.dma_start(hT_src.rearrange("(ko p) n -> p ko n", p=128), hT_sb[:])
    hT_all = dram.tile([NC, H, NB], BF16)
    nc.gpsimd.collective_compute(
        "AllGather", ALU.bypass, replica_groups=rg,
        ins=[hT_src.opt()], outs=[hT_all.opt()],
    )

    # ---- out1: o1T = relu(o1w^T @ hT_full + o1b)   [h(part,tiles), n_full]
    o1v = T["o1w"].rearrange("(ko p) m -> p ko m", p=128)
    o1T_sb = singles.tile([128, KH, N], BF16)
    hTf_sb = singles.tile([128, KH, N], BF16)
    for k in range(KH):
        nc.sync.dma_start(
            hTf_sb[:, k, :].rearrange("p (r n) -> p r n", r=NC),
            hT_all[:, 128 * k:128 * (k + 1), :].rearrange("r p n -> p r n"),
        )
    for m in range(KH):
        o1_m = wstream.tile([128, KH, 128], BF16, tag="wlhs8", name="o1_m")
        nc.sync.dma_start(o1_m[:], o1v[:, :, 128 * m:128 * (m + 1)])
        ps = mm_psum()
        for k in range(KH):
            nc.tensor.matmul(ps[:, :N], o1_m[:, k, :],
                             hTf_sb[:, k, :],
                             start=(k == 0), stop=(k == KH - 1))
        nc.scalar.activation(o1T_sb[:, m, :], ps[:, :N], AF.Relu,
                             bias=o1bT_sb[:, m:m + 1])

    # ---- out2 (vocab shard): logits[n, v] resident in SBUF bf16
    w2v = T["w2"].rearrange("(ko p) v -> p ko v", p=128)
    lg0 = logits_pool.tile([128, VS], BF16, tag="lg0")
    lg1 = logits_pool.tile([128, VS], BF16, tag="lg1")
    lg = [lg0, lg1]
    for vc in range(2 * VCH):
        w2_c = w2_pool.tile([128, KH, 256], BF16, tag="w2c", name="w2c")
        nc.sync.dma_start(w2_c[:], w2v[:, :, 256 * vc:256 * (vc + 1)])
        for mt in range(2):
            ps = mm_psum()
            for k in range(KH):
                nc.tensor.matmul(
                    ps[:, :256], o1T_sb[:, k, 128 * mt:128 * (mt + 1)],
                    w2_c[:, k, :],
                    start=(k == 0), stop=(k == KH - 1),
                )
            nc.vector.tensor_tensor(
                lg[mt][:, 256 * vc:256 * (vc + 1)], ps[:, :256],
                w2b_sb[:, 256 * vc:256 * (vc + 1)], ALU.add,
            )

    # ---- local log-softmax stats -> AllGather -> lse -> normalize
    st_src = dram.tile([4, 128], F32)
    for mt in range(2):
        rm = stat.tile([128, 1], F32, tag=f"rm{mt}", name=f"rm{mt}")
        nc.vector.tensor_reduce(rm[:], lg[mt][:], axis=AX.X, op=ALU.max)
        nrm = stat.tile([128, 1], F32, tag=f"nrm{mt}", name=f"nrm{mt}")
        nc.vector.tensor_scalar_mul(nrm[:], rm[:], -1.0)
        parts = stat.tile([128, VCH], F32, tag=f"parts{mt}", name=f"parts{mt}")
        for vc in range(VCH):
            etmp = opool.tile([128, 512], BF16, tag="etmp", name="etmp")
            nc.scalar.activation(
                etmp[:], lg[mt][:, 512 * vc:512 * (vc + 1)], AF.Exp,
                bias=nrm[:], accum_out=parts[:, vc:vc + 1],
            )
        se = stat.tile([128, 1], F32, tag=f"se{mt}", name=f"se{mt}")
        nc.vector.tensor_reduce(se[:], parts[:], axis=AX.X, op=ALU.add)
        nc.sync.dma_start(st_src[2 * mt, :, None], rm[:])
        nc.sync.dma_start(st_src[2 * mt + 1, :, None], se[:])

    st_all = dram.tile([NC, 4, 128], F32)
    nc.gpsimd.collective_compute(
        "AllGather", ALU.bypass, replica_groups=rg,
        ins=[st_src.opt()], outs=[st_all.opt()],
    )

    # ---- combine stats: lse[n] = gmax + ln(sum_r exp(max_r - gmax) * S_r)
    for mt in range(2):
        gm_all = stat.tile([128, NC], F32, tag=f"gma{mt}", name=f"gma{mt}")
        nc.sync.dma_start(gm_all[:], st_all[:, 2 * mt, :].rearrange("r p -> p r"))
        se_all = stat.tile([128, NC], F32, tag=f"sea{mt}", name=f"sea{mt}")
        nc.sync.dma_start(se_all[:],
                          st_all[:, 2 * mt + 1, :].rearrange("r p -> p r"))
        ngmax = stat.tile([128, 1], F32, tag=f"ngm{mt}", name=f"ngm{mt}")
        nc.vector.tensor_reduce(ngmax[:], gm_all[:], axis=AX.X, op=ALU.max,
                                negate=True)
        ed = stat.tile([128, NC], F32, tag=f"ed{mt}", name=f"ed{mt}")
        nc.scalar.activation(ed[:], gm_all[:], AF.Exp, bias=ngmax[:])
        nc.vector.tensor_tensor(ed[:], ed[:], se_all[:], ALU.mult)
        ssum = stat.tile([128, 1], F32, tag=f"ssum{mt}", name=f"ssum{mt}")
        nc.vector.tensor_reduce(ssum[:], ed[:], axis=AX.X, op=ALU.add)
        lns = stat.tile([128, 1], F32, tag=f"lns{mt}", name=f"lns{mt}")
        nc.scalar.activation(lns[:], ssum[:], AF.Ln)
        nlse = stat.tile([128, 1], F32, tag=f"nlse{mt}", name=f"nlse{mt}")
        nc.vector.tensor_tensor(nlse[:], ngmax[:], lns[:], ALU.subtract)
        for vc in range(VCH):
            oc = opool.tile([128, 512], F32, tag="oc", name="oc")
            nc.vector.tensor_scalar_add(oc[:], lg[mt][:, 512 * vc:512 * (vc + 1)],
                                        nlse[:])
            nc.sync.dma_start(
                T["out_lg"][128 * mt:128 * (mt + 1), 512 * vc:512 * (vc + 1)],
                oc[:],
            )

    es.close()


_CACHE = {}


def _get_nc():
    if "nc" not in _CACHE:
        _CACHE["nc"] = build_program()
    return _CACHE["nc"]


def _tile_p(v):
    return np.ascontiguousarray(np.asarray(v, dtype=np.float32).reshape(-1, 128).T)


def prep_inputs(inputs):
    """Shard + cast the full inputs into 8 per-core input maps."""
    ids = np.asarray(inputs["input_ids"])
    hidden = np.asarray(inputs["hidden"], dtype=np.float32)
    enc = np.asarray(inputs["encoder_outputs"], dtype=np.float32)
    emb = np.asarray(inputs["emb"], dtype=np.float32)
    x = emb[ids]  # [N, E] embedding row gather (pure indexing)

    f32 = lambda a: np.ascontiguousarray(a, dtype=np.float32)
    b16 = lambda a: np.ascontiguousarray(np.asarray(a, dtype=np.float32).astype(bf16))

    btanh_t = _tile_p(np.asarray(inputs["W_b"]) + np.asarray(inputs["U_b"]))
    o1bT_t = _tile_p(inputs["out1_b"])
    combbT_t = _tile_p(inputs["comb_b"])
    vT_t = _tile_p(np.asarray(inputs["v_w"], dtype=np.float32)[:, 0]).astype(bf16)
    gateb = b16(np.asarray(inputs["ih_b"]) + np.asarray(inputs["hh_b"]))[None, :]
    candb = b16(inputs["cand_b"])[None, :]
    hhcb = b16(inputs["hhc_b"])[None, :]
    lng = b16(inputs["ln_g"])[None, :]
    lnb = b16(inputs["ln_b"])[None, :]

    w2_full = np.asarray(inputs["out2_w"], dtype=np.float32)
    w2b_full = np.asarray(inputs["out2_b"], dtype=np.float32)

    sel = np.zeros((128, NB * S), dtype=np.float32)
    sel[:NB] = np.repeat(np.eye(NB, dtype=np.float32), S, axis=1)
    sel = np.ascontiguousarray(sel.astype(bf16))
    shared = {
        "sel": sel,
        "Uw": b16(inputs["U_w"]), "Ww": b16(inputs["W_w"]),
        "o1w": b16(inputs["out1_w"]), "combw": b16(inputs["comb_w"]),
        "ihw": b16(inputs["ih_w"]), "hhw": b16(inputs["hh_w"]),
        "candw": b16(inputs["cand_w"]), "hhcw": b16(inputs["hhc_w"]),
        "vT": np.ascontiguousarray(vT_t), "btanh": btanh_t,
        "o1bT": o1bT_t, "combbT": combbT_t, "gateb": gateb,
        "candb": candb, "hhcb": hhcb, "lng": lng, "lnb": lnb,
    }

    in_maps = []
    for c in range(NC):
        nb = slice(NB * c, NB * (c + 1))
        vs, ve = VS * c, min(VS * (c + 1), V)
        w2c = np.zeros((H, VS), dtype=bf16)
        w2c[:, :ve - vs] = w2_full[:, vs:ve].astype(bf16)
        w2bc = np.full((1, VS), NEG_BIG, dtype=np.float32)
        w2bc[0, :ve - vs] = w2b_full[vs:ve]
        w2bc = w2bc.astype(bf16)
        m = dict(shared)
        m["encT"] = b16(enc[:, nb, :].transpose(2, 1, 0))
        m["hidT"] = b16(hidden[nb].T)
        m["hid32"] = f32(hidden[nb])
        m["xT"] = b16(x[nb].T)
        m["w2"] = w2c
        m["w2b"] = w2bc
        in_maps.append(m)
    return in_maps


def run(inputs, trace=False):
    nc = _get_nc()
    in_maps = prep_inputs(inputs)
    res = bass_utils.run_bass_kernel_spmd(
        nc, in_maps, core_ids=list(range(NC)), trace=trace
    )
    out = np.empty((N, V), dtype=np.float32)
    h = np.empty((N, H), dtype=np.float32)
    attn = np.empty((S, N, 1), dtype=np.float32)
    for c in range(NC):
        r = res.results[c]
        vs, ve = VS * c, min(VS * (c + 1), V)
        out[:, vs:ve] = r["out_lg"][:, :ve - vs]
        h[NB * c:NB * (c + 1)] = r["out_h"]
        attn[:, NB * c:NB * (c + 1), 0] = r["out_attn"]
    return (out, h, attn), res


def kernel(**inputs):
    (out, h, attn), _ = run(inputs)
    return out, h, attn


# revision 21
# speedup vs baseline: 1.3649x; 1.1356x over previous
"""Trainium2 Bass kernel for nn_DecoderRNN (Bahdanau attention + GRU cell +
LayerNorm + vocab projection + log-softmax), SPMD over 8 NeuronCores.

Sharding: batch (N=256 -> 32/core) for attention/GRU; vocab (V=50257 ->
6656/core padded) for the H->V projection; one AllGather of the hidden state
between the phases and one tiny AllGather for the log-softmax statistics.
"""
from contextlib import ExitStack

import numpy as np
import ml_dtypes

import concourse.bass as bass
import concourse.tile as tile
from concourse import bacc, mybir
from concourse import bass_utils
from concourse.masks import make_identity

BF16 = mybir.dt.bfloat16
F32 = mybir.dt.float32
AF = mybir.ActivationFunctionType
ALU = mybir.AluOpType
AX = mybir.AxisListType

# Model dims (hardcoded per problem spec)
V, E, H, N, S = 50257, 512, 1024, 256, 128
EPS = 1e-5
NC = 8            # cores
NB = N // NC      # batch rows per core = 32
VS = 6656         # vocab shard (13 * 512), 8*6656 = 53248 >= V
VCH = VS // 512   # 13 v-chunks
KH = H // 128     # 8 k-tiles over H
KE = E // 128     # 4 k-tiles over E
SC = 8            # score chunks: (s,n) = 4096 -> 8 chunks of 512 (16 s each)
NEG_BIG = -1e30

bf16 = ml_dtypes.bfloat16


def build_program():
    nc = bacc.Bacc("TRN2", target_bir_lowering=False, debug=False, num_devices=NC)

    def din(name, shape, dt=BF16):
        return nc.dram_tensor(name, shape, dt, kind="ExternalInput").ap()

    T = {}
    # --- external inputs (per core) ---
    T["encT"] = din("encT", (SC, 128, KH, 512))      # enc chunk-major, bf16
    T["hidT"] = din("hidT", (H, NB))                 # hidden transposed, bf16
    T["hid32"] = din("hid32", (NB, H), F32)          # hidden natural, f32
    T["xT"] = din("xT", (E, NB))                     # emb rows transposed, bf16
    T["Uw"] = din("Uw", (H, H))
    T["Ww"] = din("Ww", (H, H))
    T["o1w"] = din("o1w", (H, H))
    T["combw"] = din("combw", (H + E, E))
    T["ihw"] = din("ihw", (E, 2 * H))
    T["hhw"] = din("hhw", (H, 2 * H))
    T["candw"] = din("candw", (E, H))
    T["hhcw"] = din("hhcw", (H, H))
    T["w2"] = din("w2", (H, VS))
    T["vT"] = din("vT", (128, KH))
    T["btanh"] = din("btanh", (128, KH), F32)        # W_b + U_b, tiled
    T["o1bT"] = din("o1bT", (128, KH), F32)
    T["combbT"] = din("combbT", (128, KE), F32)
    T["gateb"] = din("gateb", (1, 2 * H))       # ih_b + hh_b
    T["candb"] = din("candb", (1, H))
    T["hhcb"] = din("hhcb", (1, H))
    T["lng"] = din("lng", (1, H))
    T["lnb"] = din("lnb", (1, H))
    T["w2b"] = din("w2b", (1, VS))              # padded with -1e30
    T["sel"] = din("sel", (128, NB * S))        # Wh broadcast selector (zero-padded K)

    # --- external outputs (per core) ---
    T["out_lg"] = nc.dram_tensor("out_lg", (N, VS), F32, kind="ExternalOutput").ap()
    T["out_h"] = nc.dram_tensor("out_h", (NB, H), F32, kind="ExternalOutput").ap()
    T["out_attn"] = nc.dram_tensor("out_attn", (S, NB), F32,
                                   kind="ExternalOutput").ap()

    with tile.TileContext(nc) as tc:
        _build_tile(nc, tc, T)
    nc.compile()
    return nc


def _build_tile(nc, tc, T):
    es = ExitStack()

    def pool(name, bufs, space="SBUF"):
        return es.enter_context(tc.tile_pool(name=name, bufs=bufs, space=space))

    singles = pool("singles", 1)
    dram = pool("dram", 1, "DRAM")
    wstream = pool("wstream", 2)    # streamed single-use weights
    enc_pool = pool("enc", 2)
    ctx_pool = pool("ctxp", 2)
    tanh_pool = pool("tanh", 10)
    mm_ps = pool("mm_ps", 4, "PSUM")
    aux_ps = pool("aux_ps", 2, "PSUM")
    small = pool("small", 1)
    logits_pool = pool("logits", 1)
    w2_pool = pool("w2", 2)
    stat = pool("stat", 2)
    opool = pool("out", 2)

    rg = [list(range(NC))]

    def mm_psum():
        return mm_ps.tile([128, 512], F32, tag="mm", name="mmps")

    def aux_psum():
        return aux_ps.tile([128, 512], F32, tag="aux", name="auxps")

    # ---- collective warmup: tiny AllGather, no downstream consumers
    warm_src = dram.tile([1, 64], F32)
    warm_dst = dram.tile([NC, 64], F32)
    s_w = singles.tile([1, 64], F32, tag="warm")
    nc.vector.memset(s_w[:], 0.0)
    nc.gpsimd.dma_start(warm_src[:], s_w[:])
    nc.gpsimd.collective_compute(
        "AllGather", ALU.bypass, replica_groups=rg,
        ins=[warm_src.opt()], outs=[warm_dst.opt()],
    )

    # ---- resident constants in SBUF
    def load_sb(pool_, ap_dram, ktiles, m, dt=BF16, name="w"):
        t = pool_.tile([128, ktiles, m], dt, tag=name, name=name)
        nc.sync.dma_start(t[:], ap_dram.rearrange("(ko p) m -> p ko m", p=128))
        return t

    Uv = T["Uw"].rearrange("(ko p) m -> p ko m", p=128)
    U_sb = singles.tile([128, KH, H], BF16)
    for k in range(KH):
        nc.sync.dma_start(U_sb[:, k, :], Uv[:, k, :])

    hidT_sb = singles.tile([128, KH, NB], BF16)
    nc.sync.dma_start(hidT_sb[:], T["hidT"].rearrange("(ko p) n -> p ko n", p=128))
    hid32_sb = singles.tile([NB, H], F32)
    nc.sync.dma_start(hid32_sb[:], T["hid32"][:])
    vT_sb = singles.tile([128, KH], BF16)
    nc.sync.dma_start(vT_sb[:], T["vT"][:])
    btanh_sb = singles.tile([128, KH], F32)
    nc.sync.dma_start(btanh_sb[:], T["btanh"][:])
    o1bT_sb = singles.tile([128, KH], F32)
    nc.sync.dma_start(o1bT_sb[:], T["o1bT"][:])
    combbT_sb = singles.tile([128, KE], F32)
    nc.sync.dma_start(combbT_sb[:], T["combbT"][:])
    gateb_sb = singles.tile([NB, 2 * H], BF16)
    nc.sync.dma_start(gateb_sb[:], T["gateb"].to_broadcast((NB, 2 * H)))
    candb_sb = singles.tile([NB, H], BF16)
    nc.sync.dma_start(candb_sb[:], T["candb"].to_broadcast((NB, H)))
    hhcb_sb = singles.tile([NB, H], BF16)
    nc.sync.dma_start(hhcb_sb[:], T["hhcb"].to_broadcast((NB, H)))
    lng_sb = singles.tile([NB, H], BF16)
    nc.sync.dma_start(lng_sb[:], T["lng"].to_broadcast((NB, H)))
    lnb_sb = singles.tile([NB, H], BF16)
    nc.sync.dma_start(lnb_sb[:], T["lnb"].to_broadcast((NB, H)))
    w2b_sb = singles.tile([128, VS], BF16)
    nc.sync.dma_start(w2b_sb[:], T["w2b"].to_broadcast((128, VS)))
    ident_bf = singles.tile([128, 128], BF16)
    make_identity(nc, ident_bf[:])
    eps_sb = singles.tile([NB, 1], F32)
    nc.vector.memset(eps_sb[:], EPS)


    # ---- Wh = hidden @ W_w (natural [n, h], bf16, zero-padded to 128 rows)
    Wh_sb = singles.tile([128, H], BF16)
    nc.vector.memset(Wh_sb[:], 0.0)
    Wv = T["Ww"].rearrange("(ko p) m -> p ko m", p=128)
    for nch in range(2):
        sl = slice(512 * nch, 512 * (nch + 1))
        W_c = wstream.tile([128, KH, 512], BF16, tag="wk8", name="W_c")
        nc.sync.dma_start(W_c[:], Wv[:, :, sl])
        ps = mm_psum()
        for k in range(KH):
            nc.tensor.matmul(ps[:NB, :], hidT_sb[:, k, :], W_c[:, k, :],
                             start=(k == 0), stop=(k == KH - 1))
        nc.vector.tensor_copy(Wh_sb[:NB, sl], ps[:NB, :])

    # ---- attention: per 4-row n-chunk: scores -> softmax -> context,
    # all pipelined under the PE matmul stream.
    score_dram = dram.tile([NB, S], F32)
    attn_dram = dram.tile([NB, S], F32)
    ctxT32 = singles.tile([128, KH, NB], F32)
    identf = singles.tile([NB, NB], F32)
    make_identity(nc, identf[:])
    for c in range(SC):
        enc_c = enc_pool.tile([128, KH, 512], BF16, tag="enc_c", name="enc_c")
        nc.sync.dma_start(enc_c[:], T["encT"][c])
        sel_c = enc_pool.tile([128, 512], BF16, tag="sel_c", name="sel_c")
        nc.sync.dma_start(sel_c[:], T["sel"][:, 512 * c:512 * (c + 1)])
        ths = []
        for m in range(KH):
            ps = mm_psum()
            for k in range(KH):
                nc.tensor.matmul(
                    ps[:], U_sb[:, k, 128 * m:128 * (m + 1)], enc_c[:, k, :],
                    start=(k == 0), stop=False,
                )
            # += Wh[n] via selector matmul (K rows 32..127 are zero)
            nc.tensor.matmul(
                ps[:], Wh_sb[:, 128 * m:128 * (m + 1)], sel_c[:],
                start=False, stop=True,
            )
            th = tanh_pool.tile([128, 512], BF16, tag="th", name="th")
            nc.scalar.activation(th[:], ps[:], AF.Tanh, bias=btanh_sb[:, m:m + 1])
            ths.append(th)
        sc_ps = aux_psum()
        for m in range(KH):
            nc.tensor.matmul(
                sc_ps[:1, :], vT_sb[:, m:m + 1], ths[m][:],
                start=(m == 0), stop=(m == KH - 1),
            )
        # score chunk [1, 512] -> DRAM -> [4, 128] rows; softmax over s
        sc_sb = stat.tile([1, 512], F32, tag="sc_sb", name="sc_sb")
        nc.vector.tensor_copy(sc_sb[:], sc_ps[:1, :])
        nc.sync.dma_start(
            score_dram.rearrange("n s -> (n s)")[None, 512 * c:512 * (c + 1)],
            sc_sb[:],
        )
        sc4 = stat.tile([4, S], F32, tag="sc4", name="sc4")
        nc.sync.dma_start(sc4[:], score_dram[4 * c:4 * (c + 1), :])
        nmax4 = stat.tile([4, 1], F32, tag="nmax4", name="nmax4")
        nc.vector.tensor_reduce(nmax4[:], sc4[:], axis=AX.X, op=ALU.max,
                                negate=True)
        e4 = stat.tile([4, S], F32, tag="e4", name="e4")
        sum4 = stat.tile([4, 1], F32, tag="sum4", name="sum4")
        nc.scalar.activation(e4[:], sc4[:], AF.Exp, bias=nmax4[:],
                             accum_out=sum4[:])
        r4 = stat.tile([4, 1], F32, tag="r4", name="r4")
        nc.vector.reciprocal(r4[:], sum4[:])
        attn4 = stat.tile([4, S], F32, tag="attn4", name="attn4")
        nc.vector.tensor_scalar_mul(attn4[:], e4[:], r4[:])
        nc.sync.dma_start(attn_dram[4 * c:4 * (c + 1), :], attn4[:])
        attn_bc = ctx_pool.tile([128, 512], BF16, tag="abc", name="abc")
        nc.gpsimd.dma_start(
            attn_bc[:],
            attn_dram.rearrange("n s -> (n s)")[
                None, 512 * c:512 * (c + 1)].to_broadcast((128, 512)),
        )
        # context partial: ctxT[:, k, 4c:4c+4] = sum_s enc_c * attn
        for k in range(KH):
            wgt = ctx_pool.tile([128, 512], BF16, tag="wgt", name="wgt")
            nc.vector.tensor_tensor(wgt[:], enc_c[:, k, :], attn_bc[:], ALU.mult)
            nc.vector.tensor_reduce(
                ctxT32[:, k, 4 * c:4 * (c + 1)],
                wgt.rearrange("p (n s) -> p n s", n=4),
                axis=AX.X, op=ALU.add,
            )

    # ---- attn output: [NB, S] -> transpose on PE -> [S, NB] contiguous DMA
    attnT = small.tile([NB, S], F32, tag="attnT")
    nc.sync.dma_start(attnT[:], attn_dram[:])
    atr_ps = aux_psum()
    nc.tensor.transpose(atr_ps[:S, :NB], attnT[:], identf[:])
    attnS = small.tile([S, NB], F32, tag="attnS")
    nc.vector.tensor_copy(attnS[:], atr_ps[:S, :NB])
    nc.sync.dma_start(T["out_attn"][:], attnS[:])

    # ---- comb: gT = relu(combw^T @ [x; ctx]^T + comb_b)  [E(part,tiles), n]
    combv = T["combw"].rearrange("(ko p) m -> p ko m", p=128)
    xctxT = singles.tile([128, KH + KE, NB], BF16)
    nc.sync.dma_start(
        xctxT[:, :KE, :], T["xT"].rearrange("(ko p) n -> p ko n", p=128)
    )
    nc.vector.tensor_copy(xctxT[:, KE:, :], ctxT32[:])
    gT_sb = singles.tile([128, KE, NB], BF16)
    for m in range(KE):
        comb_m = wstream.tile([128, KH + KE, 128], BF16, tag="wlhs12",
                              name="comb_m")
        nc.sync.dma_start(comb_m[:], combv[:, :, 128 * m:128 * (m + 1)])
        ps = aux_psum()
        for k in range(KH + KE):
            nc.tensor.matmul(
                ps[:, :NB], comb_m[:, k, :], xctxT[:, k, :],
                start=(k == 0), stop=(k == KH + KE - 1),
            )
        nc.scalar.activation(gT_sb[:, m, :], ps[:, :NB], AF.Relu,
                             bias=combbT_sb[:, m:m + 1])

    # ---- GRU gates (natural layout [n, f]): sigma(g@ihw + hid@hhw + b)
    ihv = T["ihw"].rearrange("(ko p) m -> p ko m", p=128)
    hhv = T["hhw"].rearrange("(ko p) m -> p ko m", p=128)
    gates = small.tile([NB, 2 * H], F32, tag="gates")
    for nch in range(4):
        sl = slice(512 * nch, 512 * (nch + 1))
        wih = wstream.tile([128, KE, 512], BF16, tag="wk4", name="wih")
        nc.sync.dma_start(wih[:], ihv[:, :, sl])
        whh = wstream.tile([128, KH, 512], BF16, tag="wk8", name="whh")
        nc.sync.dma_start(whh[:], hhv[:, :, sl])
        ps = mm_psum()
        for k in range(KE):
            nc.tensor.matmul(ps[:NB, :], gT_sb[:, k, :], wih[:, k, :],
                             start=(k == 0), stop=False)
        for k in range(KH):
            nc.tensor.matmul(ps[:NB, :], hidT_sb[:, k, :], whh[:, k, :],
                             start=False, stop=(k == KH - 1))
        nc.vector.tensor_tensor(
            gates[:, sl], ps[:NB, :], gateb_sb[:, sl], ALU.add,
        )
    # sigmoid(x) = 0.5 * (1 + tanh(x/2))
    nc.scalar.activation(gates[:], gates[:], AF.Tanh, scale=0.5)
    nc.vector.tensor_scalar(gates[:], gates[:], 0.5, 0.5, ALU.mult, ALU.add)
    z_sl, r_sl = slice(0, H), slice(H, 2 * H)

    # ---- candidate: tanh(g@candw + cand_b + r * (hid@hhcw + hhc_b))
    hhcv = T["hhcw"].rearrange("(ko p) m -> p ko m", p=128)
    candv = T["candw"].rearrange("(ko p) m -> p ko m", p=128)
    hhc = small.tile([NB, H], F32, tag="hhc")
    for nch in range(2):
        sl = slice(512 * nch, 512 * (nch + 1))
        whhc = wstream.tile([128, KH, 512], BF16, tag="wk8", name="whhc")
        nc.sync.dma_start(whhc[:], hhcv[:, :, sl])
        ps = mm_psum()
        for k in range(KH):
            nc.tensor.matmul(ps[:NB, :], hidT_sb[:, k, :], whhc[:, k, :],
                             start=(k == 0), stop=(k == KH - 1))
        nc.vector.tensor_tensor(
            hhc[:, sl], ps[:NB, :], hhcb_sb[:, sl], ALU.add,
        )
    nc.vector.tensor_tensor(hhc[:], hhc[:], gates[:, r_sl], ALU.mult)
    cand = small.tile([NB, H], F32, tag="cand")
    for nch in range(2):
        sl = slice(512 * nch, 512 * (nch + 1))
        wcand = wstream.tile([128, KE, 512], BF16, tag="wk4", name="wcand")
        nc.sync.dma_start(wcand[:], candv[:, :, sl])
        ps = mm_psum()
        for k in range(KE):
            nc.tensor.matmul(ps[:NB, :], gT_sb[:, k, :], wcand[:, k, :],
                             start=(k == 0), stop=(k == KE - 1))
        nc.vector.tensor_tensor(cand[:, sl], ps[:NB, :], hhc[:, sl], ALU.add)
    nc.vector.tensor_tensor(cand[:], cand[:], candb_sb[:], ALU.add)
    nc.scalar.activation(cand[:], cand[:], AF.Tanh)

    # ---- h = hidden + z*(cand - hidden); LayerNorm
    hpre = small.tile([NB, H], F32, tag="hpre")
    nc.vector.tensor_tensor(hpre[:], cand[:], hid32_sb[:], ALU.subtract)
    nc.vector.tensor_tensor(hpre[:], hpre[:], gates[:, z_sl], ALU.mult)
    nc.vector.tensor_tensor(hpre[:], hpre[:], hid32_sb[:], ALU.add)
    mv = stat.tile([NB, 2, 6], F32, tag="mv", name="mv")
    nc.vector.bn_stats(mv[:, 0, :], hpre[:, 0:512])
    nc.vector.bn_stats(mv[:, 1, :], hpre[:, 512:1024])
    mu_var = stat.tile([NB, 2], F32, tag="mu_var", name="mu_var")
    nc.vector.bn_aggr(mu_var[:], mv[:])
    # rstd = exp(-0.5 * ln(var + eps))
    lnv = stat.tile([NB, 1], F32, tag="lnv", name="lnv")
    nc.scalar.activation(lnv[:], mu_var[:, 1:2], AF.Ln, bias=eps_sb[:])
    rstd = stat.tile([NB, 1], F32, tag="rstd", name="rstd")
    nc.scalar.activation(rstd[:], lnv[:], AF.Exp, scale=-0.5)
    nc.vector.tensor_scalar(hpre[:], hpre[:], mu_var[:, 0:1], rstd[:],
                            ALU.subtract, ALU.mult)
    nc.vector.tensor_tensor(hpre[:], hpre[:], lng_sb[:], ALU.mult)
    nc.vector.tensor_tensor(hpre[:], hpre[:], lnb_sb[:], ALU.add)
    nc.sync.dma_start(T["out_h"][:], hpre[:])

    # ---- transpose h -> hT (bf16), gather across cores
    hbf = small.tile([NB, H], BF16, tag="hbf")
    nc.vector.tensor_copy(hbf[:], hpre[:])
    hT_sb = singles.tile([128, KH, NB], BF16)
    for k in range(KH):
        tps = aux_ps.tile([128, NB], BF16, tag="trh", name="trh")
        nc.tensor.transpose(tps[:], hbf[:, 128 * k:128 * (k + 1)],
                            ident_bf[:NB, :NB])
        nc.vector.tensor_copy(hT_sb[:, k, :], tps[:])
    hT_src = dram.tile([H, NB], BF16)
    nc.gpsimd.dma_start(hT_src.rearrange("(ko p) n -> p ko n", p=128), hT_sb[:])
    hT_all = dram.tile([NC, H, NB], BF16)
    nc.gpsimd.collective_compute(
        "AllGather", ALU.bypass, replica_groups=rg,
        ins=[hT_src.opt()], outs=[hT_all.opt()],
    )

    # ---- out1: o1T = relu(o1w^T @ hT_full + o1b)   [h(part,tiles), n_full]
    o1v = T["o1w"].rearrange("(ko p) m -> p ko m", p=128)
    o1T_sb = singles.tile([128, KH, N], BF16)
    hTf_sb = singles.tile([128, KH, N], BF16)
    for k in range(KH):
        nc.sync.dma_start(
            hTf_sb[:, k, :].rearrange("p (r n) -> p r n", r=NC),
            hT_all[:, 128 * k:128 * (k + 1), :].rearrange("r p n -> p r n"),
        )
    for m in range(KH):
        o1_m = wstream.tile([128, KH, 128], BF16, tag="wlhs8", name="o1_m")
        nc.sync.dma_start(o1_m[:], o1v[:, :, 128 * m:128 * (m + 1)])
        ps = mm_psum()
        for k in range(KH):
            nc.tensor.matmul(ps[:, :N], o1_m[:, k, :],
                             hTf_sb[:, k, :],
                             start=(k == 0), stop=(k == KH - 1))
        nc.scalar.activation(o1T_sb[:, m, :], ps[:, :N], AF.Relu,
                             bias=o1bT_sb[:, m:m + 1])

    # ---- out2 (vocab shard): logits[n, v] resident in SBUF bf16
    w2v = T["w2"].rearrange("(ko p) v -> p ko v", p=128)
    lg0 = logits_pool.tile([128, VS], BF16, tag="lg0")
    lg1 = logits_pool.tile([128, VS], BF16, tag="lg1")
    lg = [lg0, lg1]
    cnmax = [logits_pool.tile([128, VCH], F32, tag=f"cnm{mt}", name=f"cnm{mt}")
             for mt in range(2)]
    csum = [logits_pool.tile([128, VCH], F32, tag=f"csm{mt}", name=f"csm{mt}")
            for mt in range(2)]
    for vc in range(VCH):
        w2_c = w2_pool.tile([128, KH, 512], BF16, tag="w2c", name="w2c")
        nc.sync.dma_start(w2_c[:], w2v[:, :, 512 * vc:512 * (vc + 1)])
        for mt in range(2):
            ps = mm_psum()
            for k in range(KH):
                nc.tensor.matmul(
                    ps[:], o1T_sb[:, k, 128 * mt:128 * (mt + 1)], w2_c[:, k, :],
                    start=(k == 0), stop=(k == KH - 1),
                )
            lgc = lg[mt][:, 512 * vc:512 * (vc + 1)]
            nc.vector.tensor_tensor(
                lgc, ps[:], w2b_sb[:, 512 * vc:512 * (vc + 1)], ALU.add,
            )
            # chunk-local stats (overlapped with matmul stream)
            nc.vector.tensor_reduce(cnmax[mt][:, vc:vc + 1], lgc, axis=AX.X,
                                    op=ALU.max, negate=True)
            etmp = opool.tile([128, 512], BF16, tag="etmp", name="etmp")
            nc.scalar.activation(
                etmp[:], lgc, AF.Exp,
                bias=cnmax[mt][:, vc:vc + 1], accum_out=csum[mt][:, vc:vc + 1],
            )

    # ---- combine chunk stats; ship (max, sumexp) to the stats AllGather
    st_src = dram.tile([4, 128], F32)
    for mt in range(2):
        nrm = stat.tile([128, 1], F32, tag=f"nrm{mt}", name=f"nrm{mt}")
        nc.vector.tensor_reduce(nrm[:], cnmax[mt][:], axis=AX.X, op=ALU.min)
        rm = stat.tile([128, 1], F32, tag=f"rm{mt}", name=f"rm{mt}")
        nc.vector.tensor_scalar_mul(rm[:], nrm[:], -1.0)
        dd = stat.tile([128, VCH], F32, tag=f"dd{mt}", name=f"dd{mt}")
        nc.vector.tensor_scalar(dd[:], cnmax[mt][:], nrm[:], None,
                                ALU.subtract, ALU.bypass)
        ee = stat.tile([128, VCH], F32, tag=f"ee{mt}", name=f"ee{mt}")
        nc.scalar.activation(ee[:], dd[:], AF.Exp, scale=-1.0)
        nc.vector.tensor_tensor(ee[:], ee[:], csum[mt][:], ALU.mult)
        se = stat.tile([128, 1], F32, tag=f"se{mt}", name=f"se{mt}")
        nc.vector.tensor_reduce(se[:], ee[:], axis=AX.X, op=ALU.add)
        nc.sync.dma_start(st_src[2 * mt, :, None], rm[:])
        nc.sync.dma_start(st_src[2 * mt + 1, :, None], se[:])

    st_all = dram.tile([NC, 4, 128], F32)
    nc.gpsimd.collective_compute(
        "AllGather", ALU.bypass, replica_groups=rg,
        ins=[st_src.opt()], outs=[st_all.opt()],
    )

    # ---- combine stats: lse[n] = gmax + ln(sum_r exp(max_r - gmax) * S_r)
    for mt in range(2):
        gm_all = stat.tile([128, NC], F32, tag=f"gma{mt}", name=f"gma{mt}")
        nc.sync.dma_start(gm_all[:], st_all[:, 2 * mt, :].rearrange("r p -> p r"))
        se_all = stat.tile([128, NC], F32, tag=f"sea{mt}", name=f"sea{mt}")
        nc.sync.dma_start(se_all[:],
                          st_all[:, 2 * mt + 1, :].rearrange("r p -> p r"))
        ngmax = stat.tile([128, 1], F32, tag=f"ngm{mt}", name=f"ngm{mt}")
        nc.vector.tensor_reduce(ngmax[:], gm_all[:], axis=AX.X, op=ALU.max,
                                negate=True)
        ed = stat.tile([128, NC], F32, tag=f"ed{mt}", name=f"ed{mt}")
        nc.scalar.activation(ed[:], gm_all[:], AF.Exp, bias=ngmax[:])
        nc.vector.tensor_tensor(ed[:], ed[:], se_all[:], ALU.mult)
        ssum = stat.tile([128, 1], F32, tag=f"ssum{mt}", name=f"ssum{mt}")
        nc.vector.tensor_reduce(ssum[:], ed[:], axis=AX.X, op=ALU.add)
        lns = stat.tile([128, 1], F32, tag=f"lns{mt}", name=f"lns{mt}")
        nc.scalar.activation(lns[:], ssum[:], AF.Ln)
        nlse = stat.tile([128, 1], F32, tag=f"nlse{mt}", name=f"nlse{mt}")
        nc.vector.tensor_tensor(nlse[:], ngmax[:], lns[:], ALU.subtract)
        for vc in range(VCH):
            oc = opool.tile([128, 512], F32, tag="oc", name="oc")
            nc.vector.tensor_scalar_add(oc[:], lg[mt][:, 512 * vc:512 * (vc + 1)],
                                        nlse[:])
            nc.sync.dma_start(
                T["out_lg"][128 * mt:128 * (mt + 1), 512 * vc:512 * (vc + 1)],
                oc[:],
            )

    es.close()


_CACHE = {}


def _get_nc():
    if "nc" not in _CACHE:
        _CACHE["nc"] = build_program()
    return _CACHE["nc"]


def _tile_p(v):
    return np.ascontiguousarray(np.asarray(v, dtype=np.float32).reshape(-1, 128).T)


def prep_inputs(inputs):
    """Shard + cast the full inputs into 8 per-core input maps."""
    ids = np.asarray(inputs["input_ids"])
    hidden = np.asarray(inputs["hidden"], dtype=np.float32)
    enc = np.asarray(inputs["encoder_outputs"], dtype=np.float32)
    emb = np.asarray(inputs["emb"], dtype=np.float32)
    x = emb[ids]  # [N, E] embedding row gather (pure indexing)

    f32 = lambda a: np.ascontiguousarray(a, dtype=np.float32)
    b16 = lambda a: np.ascontiguousarray(np.asarray(a, dtype=np.float32).astype(bf16))

    btanh_t = _tile_p(np.asarray(inputs["W_b"]) + np.asarray(inputs["U_b"]))
    o1bT_t = _tile_p(inputs["out1_b"])
    combbT_t = _tile_p(inputs["comb_b"])
    vT_t = _tile_p(np.asarray(inputs["v_w"], dtype=np.float32)[:, 0]).astype(bf16)
    gateb = b16(np.asarray(inputs["ih_b"]) + np.asarray(inputs["hh_b"]))[None, :]
    candb = b16(inputs["cand_b"])[None, :]
    hhcb = b16(inputs["hhc_b"])[None, :]
    lng = b16(inputs["ln_g"])[None, :]
    lnb = b16(inputs["ln_b"])[None, :]

    w2_full = np.asarray(inputs["out2_w"], dtype=np.float32)
    w2b_full = np.asarray(inputs["out2_b"], dtype=np.float32)

    sel = np.zeros((128, NB * S), dtype=np.float32)
    sel[:NB] = np.repeat(np.eye(NB, dtype=np.float32), S, axis=1)
    sel = np.ascontiguousarray(sel.astype(bf16))
    shared = {
        "sel": sel,
        "Uw": b16(inputs["U_w"]), "Ww": b16(inputs["W_w"]),
        "o1w": b16(inputs["out1_w"]), "combw": b16(inputs["comb_w"]),
        "ihw": b16(inputs["ih_w"]), "hhw": b16(inputs["hh_w"]),
        "candw": b16(inputs["cand_w"]), "hhcw": b16(inputs["hhc_w"]),
        "vT": np.ascontiguousarray(vT_t), "btanh": btanh_t,
        "o1bT": o1bT_t, "combbT": combbT_t, "gateb": gateb,
        "candb": candb, "hhcb": hhcb, "lng": lng, "lnb": lnb,
    }

    in_maps = []
    for c in range(NC):
        nb = slice(NB * c, NB * (c + 1))
        vs, ve = VS * c, min(VS * (c + 1), V)
        w2c = np.zeros((H, VS), dtype=bf16)
        w2c[:, :ve - vs] = w2_full[:, vs:ve].astype(bf16)
        w2bc = np.full((1, VS), NEG_BIG, dtype=np.float32)
        w2bc[0, :ve - vs] = w2b_full[vs:ve]
        w2bc = w2bc.astype(bf16)
        m = dict(shared)
        ec = enc[:, nb, :].reshape(S, NB, KH, 128)      # [s, n, ko, p]
        ec = ec.reshape(S, SC, 4, KH, 128).transpose(1, 4, 3, 2, 0)
        m["encT"] = b16(np.ascontiguousarray(ec).reshape(SC, 128, KH, 512))
        m["hidT"] = b16(hidden[nb].T)
        m["hid32"] = f32(hidden[nb])
        m["xT"] = b16(x[nb].T)
        m["w2"] = w2c
        m["w2b"] = w2bc
        in_maps.append(m)
    return in_maps


def run(inputs, trace=False):
    nc = _get_nc()
    in_maps = prep_inputs(inputs)
    res = bass_utils.run_bass_kernel_spmd(
        nc, in_maps, core_ids=list(range(NC)), trace=trace
    )
    out = np.empty((N, V), dtype=np.float32)
    h = np.empty((N, H), dtype=np.float32)
    attn = np.empty((S, N, 1), dtype=np.float32)
    for c in range(NC):
        r = res.results[c]
        vs, ve = VS * c, min(VS * (c + 1), V)
        out[:, vs:ve] = r["out_lg"][:, :ve - vs]
        h[NB * c:NB * (c + 1)] = r["out_h"]
        attn[:, NB * c:NB * (c + 1), 0] = r["out_attn"]
    return (out, h, attn), res


def kernel(**inputs):
    (out, h, attn), _ = run(inputs)
    return out, h, attn
